# revision 1
# baseline (speedup 1.0000x reference)
"""Trainium2 Bass kernel for per-edge dot products (u_dot_v / DotPredictor).

score[e] = dot(h[src[e]], h[dst[e]]) with h: [50000, 128] f32, src/dst: [640000] i64.

Strategy (8 NeuronCores):
  - Shard edges contiguously: 80000 edges per core. The node table h is
    replicated to every core (it stays in HBM; rows are fetched on demand).
  - Per core, both row streams (h[src], h[dst]) are fetched with the SWDGE
    `dma_gather` custom DMA instruction (512B rows from HBM into SBUF,
    edge-major layout [128, chunk, 128]).
  - `dma_gather` indices are int16, so the table is split into two halves of
    25000 rows and each core's edges are bucketed into 4 groups by
    (src_half, dst_half). Group sizes are padded to a shared per-group cap
    (max over cores, rounded to 128) so all cores run one SPMD program.
  - The per-edge dot is a fused custom DVE op (affine_mul_reduce): computes
    (hu * hv) and the 128-wide row sum in a single pass per 128-edge chunk,
    writing the product in-place over the gathered hu tile (no extra SBUF
    traffic, no same-address WAW hazards).
  - Host side: bucketing/permutation of edges (sorted by src within each
    group for HBM locality), int16 index wrapping ([128, N/16] layout the
    Q7 gather ucode expects), and inverse permutation of the returned
    scores.

  Measured on TRN2 (8 cores, trace core 0): ~393 us HW exec. The kernel is
  bound by the per-queue SWDGE serial chain: ring reclaim couples each
  queue's next descriptor generation to the previous gather's drain, so the
  ~0.7us gen + ~0.65us DGE trigger latency + ~0.9us completion-sem latency
  per gather cannot pipeline away. The 16 SDMA engines measure 27.1ns per
  512B descriptor (283us busy = 71% of span); gpsimd is 87% busy
  (gen + reclaim_for spins). Config scan on HW: TILE=896 -> 400.9us (two
  gathers fit one 128-desc ring but +14% gen fixed costs), TILE=1024 ->
  392.9us (best), TILE=1920 -> 454.9us (121/128 ring -> long reclaim
  stalls), hu+hv paired on one queue -> 462us, staggered [1152,896] tile
  pattern -> 412.9us. num_idxs is capped at 2032 by the ring (2048 hits
  illegal_instruction in reclaim_for ucode).
"""

import sys

sys.path.insert(0, "/opt/trn_rl_repo")

from contextlib import ExitStack

import numpy as np

import concourse.bacc as bacc
import concourse.bass as bass
import concourse.mybir as mybir
from concourse import library_config
from concourse.bass_utils import run_bass_kernel_spmd

N_NODES = 50000
D = 128
HALF = 25000
M = 8  # cores
# SWDGE ring carveout is 128 descriptors per engine per queue
# (dge_n_inflight=128); a dma_gather of num_idxs uses num_idxs/16 + 1
# descs/engine, so a single gather may carry at most 2032 indices (2048
# hits illegal_instruction in reclaim_for). Measured tile-size scan on HW
# (same 4-queue round-robin): 896 -> 400.9us, 1024 -> 392.9us, 1920 ->
# 454.9us; pairing a tile's hu+hv on one queue (114 descs, fits one ring)
# measured 462us. TILE=1024 with hu/hv interleaved across queues is the
# optimum: bigger tiles make the Q7 gen ucode spin longer in reclaim_for
# (ring nearly full -> gen of the next gather head-of-line blocks until the
# previous drain completes), smaller tiles pay more fixed ~1us gen costs.
TILE = 1024  # max gathered edges per DMA tile (per stream)
NQ = 4  # SWDGE queues
NBUF = 8  # gather buffer slots per stream

# group order chosen so consecutive groups share a table half where possible
GROUP_SRCS = [(0, 0), (0, 1), (1, 1), (1, 0)]  # (src_half, dst_half) per group

_cache = {}

# test harness hooks: set TRACE=True before calling kernel() to profile;
# the BassKernelResults of the last run lands in LAST_RESULTS.
TRACE = False
LAST_RESULTS = None


def _build(caps):
    """Build (and cache) the SPMD bass program for the given per-group caps."""
    key = tuple(caps)
    if key in _cache:
        return _cache[key]

    npad = int(sum(caps))
    nch = npad // 128
    ncol = npad // 16

    # tile list: split each group segment into <=TILE pieces (multiples of
    # 128). Lengths cycle through a staggered pattern so the four SWDGE
    # queues' descriptor rings do not all run dry simultaneously (each ring
    # only fits one gather; desynchronized refills hide completion latency).
    bounds = np.cumsum([0] + list(caps))
    tiles = []  # (start, length, src_half_u, src_half_v)
    pattern = [TILE]
    for g in range(4):
        su, sv = GROUP_SRCS[g]
        p = int(bounds[g])
        while p < bounds[g + 1]:
            want = pattern[len(tiles) % len(pattern)]
            ln = int(min(want, TILE, bounds[g + 1] - p))
            tiles.append((p, ln, su, sv))
            p += ln
    T = len(tiles)

    nc = bacc.Bacc("TRN2", debug=False, num_swdge_queues=NQ)
    h0 = nc.dram_tensor("h0", [HALF, D], mybir.dt.float32, kind="ExternalInput")
    h1 = nc.dram_tensor("h1", [HALF, D], mybir.dt.float32, kind="ExternalInput")
    iu = nc.dram_tensor("iu", [128, ncol], mybir.dt.int16, kind="ExternalInput")
    iv = nc.dram_tensor("iv", [128, ncol], mybir.dt.int16, kind="ExternalInput")
    score = nc.dram_tensor("score", [128, nch], mybir.dt.float32, kind="ExternalOutput")
    halves = {0: h0, 1: h1}

    with (
        nc.sbuf_tensor("iu_sb", [128, ncol], mybir.dt.int16) as iu_sb,
        nc.sbuf_tensor("iv_sb", [128, ncol], mybir.dt.int16) as iv_sb,
        nc.sbuf_tensor("hu_sb", [128, NBUF, TILE // 128, D], mybir.dt.float32) as hu_sb,
        nc.sbuf_tensor("hv_sb", [128, NBUF, TILE // 128, D], mybir.dt.float32) as hv_sb,
        nc.sbuf_tensor("score_sb", [128, nch], mybir.dt.float32) as score_sb,
        nc.semaphore("iu_h_sem") as iu_h_sem,
        nc.semaphore("iv_h_sem") as iv_h_sem,
        nc.semaphore("rest_sem") as rest_sem,
        nc.semaphore("c_sem") as c_sem,
        nc.semaphore("o_sem") as o_sem,
        ExitStack() as _stack,
        nc.Block() as block,
    ):
        # A DMA .then_inc(sem, 16) lands as 16 independent +1s (one per SDMA
        # engine), so in-flight gathers must not share a semaphore: rotate
        # per buffer slot. Reuse after NBUF tiles is safe because the gpsimd
        # c_sem wait guarantees tile t-NBUF's gather fully completed (its
        # data was consumed) before tile t's gather is issued.
        gu_sems = [_stack.enter_context(nc.semaphore(f"gu_sem{i}")) for i in range(NBUF)]
        gv_sems = [_stack.enter_context(nc.semaphore(f"gv_sem{i}")) for i in range(NBUF)]

        # idx loads are split: a small head (first 2 tiles) lets gathers
        # start ~10us earlier; the bulk arrives while tiles 0-1 drain.
        hc = min(2 * TILE // 16, ncol)

        @block.sync
        def _(sync):
            sync.dma_start(iu_sb[:, :hc], iu[:, :hc]).then_inc(iu_h_sem, 16)
            sync.dma_start(iv_sb[:, :hc], iv[:, :hc]).then_inc(iv_h_sem, 16)
            if hc < ncol:
                sync.dma_start(iu_sb[:, hc:], iu[:, hc:]).then_inc(rest_sem, 16)
                sync.dma_start(iv_sb[:, hc:], iv[:, hc:]).then_inc(rest_sem, 16)
            sync.wait_ge(c_sem, T)
            sync.dma_start(score[:], score_sb[:]).then_inc(o_sem, 16)
            sync.wait_ge(o_sem, 16)

        @block.gpsimd
        def _(gp):
            gp.load_library(library_config.mlp)
            gp.wait_ge(iu_h_sem, 16)
            gp.wait_ge(iv_h_sem, 16)
            rest_waited = hc >= ncol
            for t, (p, ln, su, sv) in enumerate(tiles):
                if not rest_waited and (p + ln) // 16 > hc:
                    gp.wait_ge(rest_sem, 32)
                    rest_waited = True
                slot = t % NBUF
                if t >= NBUF:
                    # buffer slot reusable once compute of tile t-NBUF is done
                    gp.wait_ge(c_sem, t - NBUF + 1)
                gp.dma_gather(
                    hu_sb[:, slot, : ln // 128, :],
                    halves[su][:],
                    iu_sb[:, p // 16 : (p + ln) // 16],
                    ln,
                    ln,
                    D,
                    queue_num=(2 * t) % NQ,
                    single_packet=False,
                ).then_inc(gu_sems[slot], 16)
                gp.dma_gather(
                    hv_sb[:, slot, : ln // 128, :],
                    halves[sv][:],
                    iv_sb[:, p // 16 : (p + ln) // 16],
                    ln,
                    ln,
                    D,
                    queue_num=(2 * t + 1) % NQ,
                    single_packet=False,
                ).then_inc(gv_sems[slot], 16)

        @block.vector
        def _(vec):
            for t, (p, ln, su, sv) in enumerate(tiles):
                slot = t % NBUF
                k = t // NBUF + 1
                vec.wait_ge(gu_sems[slot], 16 * k)
                vec.wait_ge(gv_sems[slot], 16 * k)
                last = None
                base = p // 128
                for c in range(ln // 128):
                    last = vec.affine_mul_reduce(
                        out=hu_sb[:, slot, c, :],
                        accum_out=score_sb[:, base + c : base + c + 1],
                        in0=hu_sb[:, slot, c, :],
                        in1=hv_sb[:, slot, c, :],
                        scale=1.0,
                        bias=0.0,
                    )
                last.then_inc(c_sem, 1)

    nc.finalize()
    _cache[key] = (nc, npad)
    return nc, npad


def _wrap_idx(vec):
    """int16 idx vector [NPAD] -> [128, NPAD/16] SWDGE layout.

    idx j lives at partition j%16, column j//16; the 16-partition block is
    replicated 8x so each Q7 core sees it in its own partition group."""
    blk = vec.reshape(-1, 16).T
    return np.ascontiguousarray(np.tile(blk, (8, 1)), dtype=np.int16)


def kernel(h=None, src=None, dst=None):
    h = np.ascontiguousarray(np.asarray(h, dtype=np.float32))
    src = np.asarray(src).astype(np.int64)
    dst = np.asarray(dst).astype(np.int64)
    E = src.shape[0]
    assert E % M == 0
    ec = E // M

    src_sh = src.reshape(M, ec)
    dst_sh = dst.reshape(M, ec)

    orders, all_counts = [], []
    for m in range(M):
        gs = (src_sh[m] >= HALF).astype(np.int64)
        gd = (dst_sh[m] >= HALF).astype(np.int64)
        gid = 2 * gs + (gs ^ gd)  # maps (0,0)->0 (0,1)->1 (1,1)->2 (1,0)->3
        # sort by src within each group: the hu gather then reads the table
        # in (nearly) ascending address order, improving HBM row locality
        order = np.lexsort((src_sh[m], gid))
        counts = np.bincount(gid, minlength=4)
        orders.append(order)
        all_counts.append(counts)
    all_counts = np.stack(all_counts)  # [M, 4]
    caps = [int(-(-int(all_counts[:, g].max()) // 128) * 128) for g in range(4)]
    caps = [max(c, 128) for c in caps]

    nc, npad = _build(caps)
    bounds = np.cumsum([0] + list(caps))

    in_maps = []
    h0 = np.ascontiguousarray(h[:HALF])
    h1 = np.ascontiguousarray(h[HALF:])
    for m in range(M):
        iu_pad = np.zeros(npad, np.int16)
        iv_pad = np.zeros(npad, np.int16)
        order, counts = orders[m], all_counts[m]
        prefix = np.cumsum(np.concatenate([[0], counts]))
        for g in range(4):
            su, sv = GROUP_SRCS[g]
            idxs = order[prefix[g] : prefix[g + 1]]
            n = len(idxs)
            b = int(bounds[g])
            iu_pad[b : b + n] = (src_sh[m][idxs] - HALF * su).astype(np.int16)
            iv_pad[b : b + n] = (dst_sh[m][idxs] - HALF * sv).astype(np.int16)
        in_maps.append(
            {"h0": h0, "h1": h1, "iu": _wrap_idx(iu_pad), "iv": _wrap_idx(iv_pad)}
        )

    res = run_bass_kernel_spmd(nc, in_maps, core_ids=list(range(M)), trace=TRACE)
    global LAST_RESULTS
    LAST_RESULTS = res

    out = np.empty(E, np.float32)
    for m in range(M):
        vec = res.results[m]["score"].T.reshape(-1)  # padded pos = c*128+p
        order, counts = orders[m], all_counts[m]
        prefix = np.cumsum(np.concatenate([[0], counts]))
        for g in range(4):
            n = int(counts[g])
            b = int(bounds[g])
            out[m * ec + order[prefix[g] : prefix[g] + n]] = vec[b : b + n]
    return out



# revision 3
# speedup vs baseline: 1.1879x; 1.1879x over previous
"""Trainium2 Bass kernel v2 for per-edge dot products (u_dot_v / DotPredictor).

score[e] = dot(h[src[e]], h[dst[e]]), h: [50000, 128] f32, src/dst: [640000] i64.

Strategy (8 NeuronCores), v2 = src-octant sharding + PE one-hot expansion:
  - Edges are assigned to the core owning their SRC octant (6250 nodes each).
    The hu side then reads only a 3.2MB table slice, which is SBUF-resident
    as bf16 hi/mid/lo (exact 3-way split of f32), so hu rows are produced
    ON-CHIP by PE one-hot matmuls instead of HBM gathers. Only the hv side
    (dst rows, random over the full table) is SWDGE-gathered from HBM.
    This cuts HBM gather traffic per core from ~82MB to ~46MB.
  - Per core, edges are grouped by (dst-half, src-window of 128 rows) with
    per-(half,window) caps shared across cores (single SPMD program), sorted
    by dst within each group for HBM locality.
  - Per 512-edge chunk: a K=3 basis matmul ([srcpos, srcpos^2-hi, srcpos^2-lo]
    all bf16-exact) replicates rep2[u,e] = srcpos^2 - 2*u*srcpos across
    partitions in PSUM; ONE Scalar-engine Relu (bias 1-u^2, scale -1) turns it
    into the exact bf16 one-hot relu(1-(srcpos-u)^2). Per 128-edge subtile,
    three accumulating bf16 matmuls (onehot as stationary, slice_{hi,mid,lo}
    moving) rebuild the exact f32 hu rows in PSUM; DVE affine_mul_reduce
    multiplies with the gathered hv tile and reduces over features into the
    score column. Rep matmuls are issued 3 chunks ahead (3 PSUM banks) so the
    PE->ACT->PE loop pipelines; all sync is chunk-granular.

  Measured on TRN2 (8 cores): ~292 us HW exec, exact (rel err 0.0) vs the
  f32 reference. Baseline v1 (SWDGE-gather both endpoints): ~390 us,
  SDMA-bound at ~32.8 ns per 512B random-row descriptor. v2 engine busy:
  DVE ~96% (4 affine_mul_reduce/chunk, ~300 ns each incl PSUM-read init),
  PE ~89% (12 MM + rep per chunk), ACT ~50%, SDMA ~75%. Chunk cycle
  ~1.3 us paced by the PE/DVE pair plus inter-engine sem latency.
  Tried and rejected: rep-ahead 4 + deeper one-hot ring (336 us, PSUM
  bank pressure); fp16 2-way slice split (passes at rel err 0.0154 but
  only 30% margin under the 2e-2 gate; helps only PE which is not the
  pacer); per-subtile DVE is_equal one-hot (226 us DVE, too slow);
  tensor_mask_reduce extraction of G^T needs hv transposed (impossible:
  PE has no PSUM read and f32 transposed gather is unsupported).
"""

import sys

sys.path.insert(0, "/opt/trn_rl_repo")

from contextlib import ExitStack

import ml_dtypes
import numpy as np

import concourse.bacc as bacc
import concourse.bass as bass
import concourse.mybir as mybir
from concourse import library_config
from concourse.bass_utils import run_bass_kernel_spmd

BF16 = np.dtype(ml_dtypes.bfloat16)

N_NODES = 50000
D = 128
HALF = 25000
M = 8  # cores
SLICE = N_NODES // M  # 6250 nodes per core
NW = (SLICE + 127) // 128  # 49 windows of 128 nodes
TILE = 1536  # hv gather tile (12 subtiles = 3 chunks); <= 2032 ring cap, mult of 512
NQ = 4  # SWDGE queues
NBUF = 8  # hv gather buffer slots (multiple of NQ: slot i always serves queue i%NQ)
NOHC = 4  # one-hot ring slots (chunks of 512 edges)
NHU = 4  # psum hu chunk slots ([128, 4, 128] each)
SLICE_GROUPS = 4

_cache = {}

TRACE = False
LAST_RESULTS = None


def _window_group(w):
    return min(w * SLICE_GROUPS // NW, SLICE_GROUPS - 1)


def _build(caps):
    """Build the SPMD bass program for per-(half,window) caps [2*NW]."""
    key = tuple(caps)
    if key in _cache:
        return _cache[key]

    caps = np.asarray(caps, dtype=np.int64).reshape(2, NW)
    npad = int(caps.sum())
    S = npad // 128  # subtiles
    ncol = npad // 16  # iv idx columns
    nchunk = (npad + 511) // 512  # srcpos replication chunks
    CH4 = (nchunk + 2) // 3  # srcpos chunk columns (3 partition stripes)

    # subtile -> (half, window); gather tiles within each half run
    sub_w = []  # window of each subtile
    seg_half = []
    bounds = [0]
    for h in range(2):
        for w in range(NW):
            c = int(caps[h, w])
            for _ in range(c // 128):
                sub_w.append(w)
                seg_half.append(h)
            bounds.append(bounds[-1] + c)
    assert len(sub_w) == S

    run_len = [int(caps[0].sum()), int(caps[1].sum())]
    tiles = []  # (start, len, half)
    p = 0
    for h in range(2):
        end = p + run_len[h]
        while p < end:
            ln = int(min(TILE, end - p))
            tiles.append((p, ln, h))
            p += ln
    T = len(tiles)
    # subtile -> tile index and local chunk offset
    sub_tile = np.zeros(S, np.int64)
    sub_loc = np.zeros(S, np.int64)
    for t, (p, ln, h) in enumerate(tiles):
        for j in range(ln // 128):
            sub_tile[(p // 128) + j] = t
            sub_loc[(p // 128) + j] = j
    tile_end_sub = [(p + ln) // 128 for (p, ln, h) in tiles]

    nc = bacc.Bacc("TRN2", debug=False, num_swdge_queues=NQ)
    h0 = nc.dram_tensor("h0", [HALF, D], mybir.dt.float32, kind="ExternalInput")
    h1 = nc.dram_tensor("h1", [HALF, D], mybir.dt.float32, kind="ExternalInput")
    iv = nc.dram_tensor("iv", [128, ncol], mybir.dt.int16, kind="ExternalInput")
    srcpos = nc.dram_tensor("srcpos", [128, CH4, 512], mybir.dt.bfloat16, kind="ExternalInput")
    basis = nc.dram_tensor("basis", [128, 128], mybir.dt.bfloat16, kind="ExternalInput")
    iota = nc.dram_tensor("iota", [128, 1], mybir.dt.float32, kind="ExternalInput")
    sl_hi = nc.dram_tensor("sl_hi", [128, NW, 128], mybir.dt.bfloat16, kind="ExternalInput")
    sl_mid = nc.dram_tensor("sl_mid", [128, NW, 128], mybir.dt.bfloat16, kind="ExternalInput")
    sl_lo = nc.dram_tensor("sl_lo", [128, NW, 128], mybir.dt.bfloat16, kind="ExternalInput")
    score = nc.dram_tensor("score", [128, S], mybir.dt.float32, kind="ExternalOutput")
    halves = {0: h0, 1: h1}

    # slice group column ranges
    gcols = [[w for w in range(NW) if _window_group(w) == g] for g in range(SLICE_GROUPS)]
    gbnd = [0]
    for g in range(SLICE_GROUPS):
        gbnd.append(gbnd[-1] + len(gcols[g]))

    with ExitStack() as _stack:
        ec = _stack.enter_context
        iv_sb = ec(nc.sbuf_tensor("iv_sb", [128, ncol], mybir.dt.int16))
        srcpos_sb = ec(nc.sbuf_tensor("srcpos_sb", [128, CH4, 512], mybir.dt.bfloat16))
        basis_sb = ec(nc.sbuf_tensor("basis_sb", [128, 128], mybir.dt.bfloat16))
        iota_sb = ec(nc.sbuf_tensor("iota_sb", [128, 1], mybir.dt.float32))
        slhi_sb = ec(nc.sbuf_tensor("slhi_sb", [128, NW, 128], mybir.dt.bfloat16))
        slmid_sb = ec(nc.sbuf_tensor("slmid_sb", [128, NW, 128], mybir.dt.bfloat16))
        sllo_sb = ec(nc.sbuf_tensor("sllo_sb", [128, NW, 128], mybir.dt.bfloat16))
        hv_sb = ec(nc.sbuf_tensor("hv_sb", [128, NBUF, TILE // 128, D], mybir.dt.float32))
        oh_sb = ec(nc.sbuf_tensor("oh_sb", [128, NOHC, 512], mybir.dt.bfloat16))
        score_sb = ec(nc.sbuf_tensor("score_sb", [128, S], mybir.dt.float32))
        rep_ps = [ec(nc.psum_tensor(f"rep_ps{i}", [128, 512], mybir.dt.float32)) for i in range(3)]
        hu_ps = [ec(nc.psum_tensor(f"hu_ps{i}", [128, 4, 128], mybir.dt.float32)) for i in range(NHU)]
        const_sem = ec(nc.semaphore("const_sem"))
        slice_sems = [ec(nc.semaphore(f"slice_sem{g}")) for g in range(SLICE_GROUPS)]
        iv_h_sem = ec(nc.semaphore("iv_h_sem"))
        rest_sem = ec(nc.semaphore("rest_sem"))
        rep_sem = ec(nc.semaphore("rep_sem"))
        oh_sem = ec(nc.semaphore("oh_sem"))
        hu_sem = ec(nc.semaphore("hu_sem"))
        c_sem = ec(nc.semaphore("c_sem"))
        o_sem = ec(nc.semaphore("o_sem"))
        gu_sems = [ec(nc.semaphore(f"gu_sem{i}")) for i in range(NBUF)]
        block = ec(nc.Block())

        hc = min(2 * TILE // 16, ncol)  # idx head columns (first 2 tiles)

        @block.sync
        def _(sync):
            sync.dma_start(iv_sb[:, :hc], iv[:, :hc]).then_inc(iv_h_sem, 16)
            sync.dma_start(srcpos_sb[:], srcpos[:]).then_inc(const_sem, 16)
            sync.dma_start(basis_sb[:], basis[:]).then_inc(const_sem, 16)
            sync.dma_start(iota_sb[:], iota[:]).then_inc(const_sem, 16)
            # slice group 0 before idx bulk: PE needs it first
            a, b = gbnd[0], gbnd[1]
            sync.dma_start(slhi_sb[:, a:b, :], sl_hi[:, a:b, :]).then_inc(slice_sems[0], 16)
            sync.dma_start(slmid_sb[:, a:b, :], sl_mid[:, a:b, :]).then_inc(slice_sems[0], 16)
            sync.dma_start(sllo_sb[:, a:b, :], sl_lo[:, a:b, :]).then_inc(slice_sems[0], 16)
            if hc < ncol:
                sync.dma_start(iv_sb[:, hc:], iv[:, hc:]).then_inc(rest_sem, 16)
            for g in range(1, SLICE_GROUPS):
                a, b = gbnd[g], gbnd[g + 1]
                sync.dma_start(slhi_sb[:, a:b, :], sl_hi[:, a:b, :]).then_inc(slice_sems[g], 16)
                sync.dma_start(slmid_sb[:, a:b, :], sl_mid[:, a:b, :]).then_inc(slice_sems[g], 16)
                sync.dma_start(sllo_sb[:, a:b, :], sl_lo[:, a:b, :]).then_inc(slice_sems[g], 16)
            sync.wait_ge(c_sem, S)
            sync.dma_start(score[:], score_sb[:]).then_inc(o_sem, 16)
            sync.wait_ge(o_sem, 16)

        @block.gpsimd
        def _(gp):
            gp.load_library(library_config.mlp)
            gp.wait_ge(iv_h_sem, 16)
            rest_waited = hc >= ncol
            for t, (p, ln, hf) in enumerate(tiles):
                if not rest_waited and (p + ln) // 16 > hc:
                    gp.wait_ge(rest_sem, 16)
                    rest_waited = True
                slot = t % NBUF
                if t >= NBUF:
                    gp.wait_ge(c_sem, tile_end_sub[t - NBUF])
                gp.dma_gather(
                    hv_sb[:, slot, : ln // 128, :],
                    halves[hf][:],
                    iv_sb[:, p // 16 : (p + ln) // 16],
                    ln,
                    ln,
                    D,
                    queue_num=t % NQ,
                    single_packet=False,
                ).then_inc(gu_sems[slot], 16)

        @block.tensor
        def _(tensor):
            tensor.wait_ge(const_sem, 48)
            g_seen = -1

            def emit_rep(c):
                ln = min(512, npad - 512 * c)
                q = 32 * (c % 3)
                tensor.matmul(
                    out=rep_ps[c % 3][:, :ln],
                    lhsT=basis_sb[q : q + 3, :],
                    rhs=srcpos_sb[q : q + 3, c // 3, :ln],
                    start=True,
                    stop=True,
                ).then_inc(rep_sem, 1)

            for c in range(min(3, nchunk)):
                emit_rep(c)
            for c in range(nchunk):
                ln = min(512, npad - 512 * c)
                tensor.wait_ge(oh_sem, c + 1)
                if c >= NHU:
                    tensor.wait_ge(c_sem, 4 * (c - NHU) + 4)
                nsub = ln // 128
                for j in range(nsub):
                    s_ = 4 * c + j
                    w = sub_w[s_]
                    g = _window_group(w)
                    while g > g_seen:
                        g_seen += 1
                        tensor.wait_ge(slice_sems[g_seen], 48)
                    last = None
                    for k, sl in enumerate((slhi_sb, slmid_sb, sllo_sb)):
                        last = tensor.matmul(
                            out=hu_ps[c % NHU][:, j, :],
                            lhsT=oh_sb[:, c % NOHC, 128 * j : 128 * j + 128],
                            rhs=sl[:, w, :],
                            start=(k == 0),
                            stop=(k == 2),
                        )
                    if j == nsub - 1:
                        last.then_inc(hu_sem, 1)
                if c + 3 < nchunk:
                    # rep for chunk c+3 reuses bank c%3: safe, ACT finished
                    # chunk c (oh_sem >= c+1 waited above)
                    emit_rep(c + 3)

        @block.scalar
        def _(act):
            act.wait_ge(const_sem, 48)
            for c in range(nchunk):
                ln = min(512, npad - 512 * c)
                act.wait_ge(rep_sem, c + 1)
                if c >= NOHC:
                    act.wait_ge(hu_sem, c - NOHC + 1)
                # onehot[u,e] = relu(1 - (srcpos[e]-u)^2), rep2 = srcpos^2 - 2u*srcpos
                act.activation(
                    oh_sb[:, c % NOHC, :ln],
                    rep_ps[c % 3][:, :ln],
                    mybir.ActivationFunctionType.Relu,
                    bias=iota_sb[:, 0:1],
                    scale=-1.0,
                ).then_inc(oh_sem, 1)

        LAG = 2

        @block.vector
        def _(vec):
            cur_tile = -1
            cur_chunk = -1
            for j in range(S):
                c = j // 4
                t = int(sub_tile[j])
                if t != cur_tile:
                    vec.wait_ge(gu_sems[t % NBUF], 16 * (t // NBUF + 1))
                    cur_tile = t
                if c != cur_chunk:
                    vec.wait_ge(hu_sem, c + 1)
                    cur_chunk = c
                slot = t % NBUF
                vec.affine_mul_reduce(
                    out=hv_sb[:, slot, int(sub_loc[j]), :],
                    accum_out=score_sb[:, j : j + 1],
                    in0=hv_sb[:, slot, int(sub_loc[j]), :],
                    in1=hu_ps[c % NHU][:, j % 4, :],
                    scale=1.0,
                    bias=0.0,
                ).then_inc(c_sem, 1)

    nc.finalize()
    _cache[key] = (nc, npad, bounds)
    return nc, npad, bounds


def _wrap_idx(vec):
    """int16 idx vector [NPAD] -> [128, NPAD/16] SWDGE layout (replicated 8x)."""
    blk = vec.reshape(-1, 16).T
    return np.ascontiguousarray(np.tile(blk, (8, 1)), dtype=np.int16)


def _split3(x):
    """Exact 3-way bf16 split of f32 array: x ~= hi + mid + lo."""
    hi = x.astype(BF16)
    r = x - hi.astype(np.float32)
    mid = r.astype(BF16)
    r2 = r - mid.astype(np.float32)
    lo = r2.astype(BF16)
    return hi, mid, lo


def kernel(h=None, src=None, dst=None):
    h = np.ascontiguousarray(np.asarray(h, dtype=np.float32))
    src = np.asarray(src).astype(np.int64)
    dst = np.asarray(dst).astype(np.int64)
    E = src.shape[0]

    oct_ = src // SLICE
    orders, metas = [], []
    counts = np.zeros((M, 2, NW), np.int64)
    for m in range(M):
        eidx = np.nonzero(oct_ == m)[0]
        srcl = src[eidx] - SLICE * m
        w = srcl >> 7
        half = (dst[eidx] >= HALF).astype(np.int64)
        order = np.lexsort((dst[eidx], w, half))
        orders.append((eidx, order))
        cnt = np.bincount((half * NW + w), minlength=2 * NW).reshape(2, NW)
        counts[m] = cnt
        metas.append((srcl, w, half))
    caps = np.maximum(counts.max(axis=0), 0)
    caps = ((caps + 127) // 128) * 128  # per-(half,window) caps, mult of 128
    for hh in range(2):
        rem = int(caps[hh].sum()) % 512
        if rem:
            caps[hh, NW - 1] += 512 - rem  # half-run sums mult of 512 (chunk/tile align)

    nc, npad, bounds = _build(caps.reshape(-1))
    S = npad // 128
    nchunk = (npad + 511) // 512
    CH4 = (nchunk + 2) // 3

    # constants shared by all cores
    u = np.arange(128, dtype=np.float32)
    basis = np.zeros((128, 128), BF16)
    for q in (0, 32, 64):
        basis[q, :] = (-2.0 * u).astype(BF16)
        basis[q + 1, :] = 256.0
        basis[q + 2, :] = 1.0
    iota_col = (1.0 - u * u).reshape(128, 1).astype(np.float32)

    h0 = np.ascontiguousarray(h[:HALF])
    h1 = np.ascontiguousarray(h[HALF:])

    in_maps = []
    caps_flat = caps.reshape(-1)
    for m in range(M):
        eidx, order = orders[m]
        srcl, w, half = metas[m]
        iv_pad = np.zeros(npad, np.int16)
        sp_pad = np.zeros(npad, np.float32)
        prefix = np.zeros(2 * NW + 1, np.int64)
        cnt_flat = counts[m].reshape(-1)
        prefix[1:] = np.cumsum(cnt_flat)
        pos = 0
        for k in range(2 * NW):
            n = int(cnt_flat[k])
            b = bounds[k]
            idxs = order[prefix[k] : prefix[k] + n]
            iv_pad[b : b + n] = (dst[eidx[idxs]] - HALF * (k // NW)).astype(np.int16)
            sp_pad[b : b + n] = (srcl[idxs] & 127).astype(np.float32)
            pos += n
        # srcpos chunks -> [128, CH4, 512] bf16 on partition stripes 0/32/64/96
        sp_sb = np.zeros((128, CH4, 512), BF16)
        sq = sp_pad * sp_pad
        s_hi = np.floor(sq / 256.0)
        s_lo = sq - 256.0 * s_hi
        for c in range(nchunk):
            ln = min(512, npad - 512 * c)
            q = 32 * (c % 3)
            sp_sb[q, c // 3, :ln] = sp_pad[512 * c : 512 * c + ln].astype(BF16)
            sp_sb[q + 1, c // 3, :ln] = s_hi[512 * c : 512 * c + ln].astype(BF16)
            sp_sb[q + 2, c // 3, :ln] = s_lo[512 * c : 512 * c + ln].astype(BF16)

        # per-core table slice, padded to NW*128 rows, node-major [128, NW, 128]
        sub = np.zeros((NW * 128, D), np.float32)
        sub[:SLICE] = h[SLICE * m : SLICE * (m + 1)]
        hi, mid, lo = _split3(sub)
        sl3 = [
            np.ascontiguousarray(a.reshape(NW, 128, D).transpose(1, 0, 2))
            for a in (hi, mid, lo)
        ]

        in_maps.append(
            {
                "h0": h0,
                "h1": h1,
                "iv": _wrap_idx(iv_pad),
                "srcpos": sp_sb,
                "basis": basis,
                "iota": iota_col,
                "sl_hi": sl3[0],
                "sl_mid": sl3[1],
                "sl_lo": sl3[2],
            }
        )

    res = run_bass_kernel_spmd(nc, in_maps, core_ids=list(range(M)), trace=TRACE)
    global LAST_RESULTS
    LAST_RESULTS = res

    out = np.empty(E, np.float32)
    for m in range(M):
        eidx, order = orders[m]
        vec = res.results[m]["score"].T.reshape(-1)  # padded pos = s*128 + p
        cnt_flat = counts[m].reshape(-1)
        prefix = np.zeros(2 * NW + 1, np.int64)
        prefix[1:] = np.cumsum(cnt_flat)
        for k in range(2 * NW):
            n = int(cnt_flat[k])
            b = bounds[k]
            out[eidx[order[prefix[k] : prefix[k] + n]]] = vec[b : b + n]
    return out
